# revision 27
# baseline (speedup 1.0000x reference)
"""Trainium2 Bass kernel for nn_CommunicationLayer (gnn_message_passing).

Computes, for A=3 agents over batch B with feature dim D=128:
    total       = sum_a x_a                      # [1, B, D]
    mean_others = (total - x_i) / (A-1)          # [A, B, D]
    out_i       = x_i + mean_others_i @ W + b    # [A, B, D]

Rewritten as   out_i = sum_j x_j @ M_ji + b,  M_ii = I, M_ij = W/(A-1)
so the whole computation is 3 accumulating matmuls per 128-row batch tile
(residual included via the identity diagonal blocks); no total/mean tensors
are ever materialized and no on-chip transposes are needed: the host passes
x pre-transposed to feature-major, with the within-chunk column order chosen
so PSUM partition order matches the batch-major store layout exactly.

Distribution: data-parallel over the batch axis across 8 NeuronCores
(no cross-device communication), weights replicated.

Per-core dataflow (chunks of 2048 batch rows, 3 MiB DMAs, 8 KiB runs):
  DMA in (feature-major x, f32r)
    -> 3x f32r matmul per group, rhs = [M_j0|M_j1|M_j2] (N=384 -> 1 cyc/row)
       accumulating into one PSUM bank
    -> PSUM->SBUF evacuation alternating DVE / ACT
    -> DMA out (batch-major, issued on GPSIMD so the SP load stream never
       blocks behind store data dependencies)
"""

import numpy as np

import concourse.bacc as bacc
import concourse.bass as bass  # noqa: F401  (kept for users poking at internals)
import concourse.mybir as mybir
from concourse.tile import TileContext
from concourse.bass_utils import run_bass_kernel_spmd

A = 3
B = 524288
D = 128
NCORES = 8
BC = B // NCORES          # 65536 batch rows per core
CHUNK = 2048              # batch rows per chunk
W_PER = CHUNK // 128      # 16 column-blocks (groups) per chunk
NCHUNK = BC // CHUNK      # 32

F32 = mybir.dt.float32
F32R = mybir.dt.float32r


def build_bass():
    # Bacc (not plain Bass): its compile pipeline moves matmul waits onto
    # ldweights and splits >1-wait sync conditions into event semaphores,
    # which the fused 4-byte matmuls need to pass walrus codegen.
    nc = bacc.Bacc(None, target_bir_lowering=False)

    # x arrives feature-major ([A, D, BC]) and column-permuted (see run()),
    # declared float32r so the PE matmul runs the 1 cycle/row fast path.
    x_ext = nc.declare_dram_parameter("x", [A, D, BC], F32R, isOutput=False)
    m_ext = nc.declare_dram_parameter("m", [D, A * A * D], F32, isOutput=False)
    y_ext = nc.declare_dram_parameter("y", [A, BC, D], F32, isOutput=True)

    with TileContext(nc) as tc:
        with (
            tc.tile_pool(name="const", bufs=1) as cpool,
            tc.tile_pool(name="xin_pool", bufs=5) as in_pool,
            tc.tile_pool(name="xout_pool", bufs=4) as out_pool,
            tc.tile_pool(name="mpsum_pool", bufs=8, space="PSUM") as mpsum_pool,
        ):
            mw_f = cpool.tile([D, A * A * D], F32)
            nc.sync.dma_start(out=mw_f, in_=m_ext[:, :])
            # Walrus requires f32r matmul operands to be produced as f32r;
            # the ACT copy performs the rounding cast.
            mw_r = cpool.tile([D, A * A * D], F32R)
            nc.scalar.copy(out=mw_r, in_=mw_f)

            for c in range(NCHUNK):
                b0 = c * CHUNK
                # [128 (d), A * 2048]; per partition: one 8 KiB contiguous
                # DRAM run per agent.
                xin = in_pool.tile([128, A * CHUNK], F32R, tag="xin")
                src = x_ext[:, :, b0:b0 + CHUNK].rearrange("a d w -> d a w")
                nc.sync.dma_start(
                    out=xin.rearrange("p (a f) -> p a f", a=A), in_=src
                )

                for h in range(2):
                    # Per-half-chunk output tile: its store DMA (issued on
                    # the otherwise-idle GPSIMD sequencer) waits only on this
                    # half's 8 evacuations, so the SP sequencer's load stream
                    # never blocks behind store data dependencies, and
                    # stores start draining early.
                    xoh = out_pool.tile([128, A * 8 * D], F32, tag="xout")
                    xoh4 = xoh.rearrange("p (a w d) -> p a w d", a=A, d=D)

                    for w4 in range(8):
                        w = 8 * h + w4
                        ps = mpsum_pool.tile([128, A * D], F32, tag="ps")
                        ps_r = ps.rearrange("p (i d) -> p i d", d=D)
                        for j in range(A):
                            nc.tensor.matmul(
                                ps,
                                lhsT=xin[:, j * CHUNK + w * D:
                                         j * CHUNK + (w + 1) * D],
                                rhs=mw_r[:, j * A * D:(j + 1) * A * D],
                                start=(j == 0),
                                stop=(j == A - 1),
                            )
                        # PSUM -> SBUF evacuation, alternating DVE / ACT to
                        # split the copy bandwidth across both engines.
                        if w4 % 2 == 0:
                            nc.vector.tensor_copy(
                                out=xoh4[:, :, w4, :], in_=ps_r
                            )
                        else:
                            nc.scalar.copy(out=xoh4[:, :, w4, :], in_=ps_r)

                    dst = y_ext[:, b0:b0 + CHUNK, :].rearrange(
                        "a (p w) d -> p a w d", p=128
                    )[:, :, 8 * h:8 * h + 8, :]
                    nc.gpsimd.dma_start(out=dst, in_=xoh4)

    # Bacc defers register allocation to its compile() pass (run by
    # finalize); the PJRT exec path serializes nc as-is, so finalize here.
    nc.finalize()
    return nc


def _prep_shard(shard):
    """[A, BC, D] batch-major -> [A, D, BC] feature-major with the
    within-chunk column order (c, w, k) -> batch row c*2048 + k*16 + w,
    so that matmul PSUM partition order (k) matches the batch-major store
    layout (partition k holds rows b0 + 16k + w)."""
    xs = shard.reshape(A, NCHUNK, 128, W_PER, D)      # [a, c, k, w, d]
    xth = xs.transpose(0, 4, 1, 3, 2)                  # [a, d, c, w, k]
    return np.ascontiguousarray(xth.reshape(A, D, BC))


def run(inputs, trace=False):
    """Build, compile, and run on 8 cores. Returns (full_output, results_obj)."""
    agent_states = np.asarray(inputs["agent_states"], dtype=np.float32)
    W = np.asarray(inputs["W"], dtype=np.float32)
    b = np.asarray(inputs["b"], dtype=np.float32)

    wp = (W * (1.0 / (A - 1))).astype(np.float32)
    eye = np.eye(D, dtype=np.float32)
    # m[:, j*A*D + i*D : ...] = M_ji  (I on the diagonal -> residual x_i)
    blocks = []
    for j in range(A):
        for i in range(A):
            blocks.append(eye if i == j else wp)
    m_host = np.ascontiguousarray(np.concatenate(blocks, axis=1))

    nc = build_bass()

    in_maps = []
    for i in range(NCORES):
        shard = np.ascontiguousarray(agent_states[:, i * BC:(i + 1) * BC, :])
        in_maps.append({"x": _prep_shard(shard), "m": m_host})

    res = run_bass_kernel_spmd(nc, in_maps, list(range(NCORES)), trace=trace)

    out = np.concatenate([r["y"] for r in res.results], axis=1)
    if np.any(b):
        out = out + b.reshape(1, 1, D)
    return out, res


def kernel(**inputs):
    out, _ = run(inputs, trace=False)
    return out


# revision 30
# speedup vs baseline: 1.2255x; 1.2255x over previous
"""Trainium2 Bass kernel for nn_CommunicationLayer (gnn_message_passing).

Computes, for A=3 agents over batch B with feature dim D=128:
    total       = sum_a x_a                      # [1, B, D]
    mean_others = (total - x_i) / (A-1)          # [A, B, D]
    out_i       = x_i + mean_others_i @ W + b    # [A, B, D]

Rewritten as   out_i = sum_j x_j @ M_ji + b,  M_ii = I, M_ij = W/(A-1)
so the whole computation is 3 accumulating matmuls per 128-row batch tile
(residual included via the identity diagonal blocks); no total/mean tensors
are ever materialized and no on-chip transposes are needed: the host passes
x pre-transposed to feature-major, with the within-chunk column order chosen
so PSUM partition order matches the batch-major store layout exactly.

Distribution: data-parallel over the batch axis across 8 NeuronCores
(no cross-device communication), weights replicated.

Per-core dataflow (chunks of 2048 batch rows, 3 MiB DMAs, 8 KiB runs):
  DMA in (feature-major x, f32r)
    -> 3x f32r matmul per group, rhs = [M_j0|M_j1|M_j2] (N=384 -> 1 cyc/row)
       accumulating into one PSUM bank
    -> PSUM->SBUF evacuation alternating DVE / ACT
    -> DMA out (batch-major, issued on GPSIMD so the SP load stream never
       blocks behind store data dependencies)
"""

import numpy as np

import concourse.bacc as bacc
import concourse.bass as bass  # noqa: F401  (kept for users poking at internals)
import concourse.mybir as mybir
from concourse.tile import TileContext
from concourse.bass_utils import run_bass_kernel_spmd

A = 3
B = 524288
D = 128
NCORES = 8
BC = B // NCORES          # 65536 batch rows per core
CHUNK = 2048              # batch rows per chunk
W_PER = CHUNK // 128      # 16 column-blocks (groups) per chunk
NCHUNK = BC // CHUNK      # 32

F32 = mybir.dt.float32
F32R = mybir.dt.float32r


def build_bass():
    # Bacc (not plain Bass): its compile pipeline moves matmul waits onto
    # ldweights and splits >1-wait sync conditions into event semaphores,
    # which the fused 4-byte matmuls need to pass walrus codegen.
    nc = bacc.Bacc(None, target_bir_lowering=False)

    # x arrives feature-major, chunk-major ([NCHUNK, A, D, CHUNK]) and
    # column-permuted (see run()) — each chunk is A dense 1 MiB DRAM
    # regions, so load descriptors stay HBM-local. Declared float32r so
    # the PE matmul runs the 1 cycle/row fast path.
    x_ext = nc.declare_dram_parameter(
        "x", [NCHUNK, A, D, CHUNK], F32R, isOutput=False
    )
    m_ext = nc.declare_dram_parameter("m", [D, A * A * D], F32, isOutput=False)
    y_ext = nc.declare_dram_parameter("y", [A, BC, D], F32, isOutput=True)

    with TileContext(nc) as tc:
        with (
            tc.tile_pool(name="const", bufs=1) as cpool,
            tc.tile_pool(name="xin_pool", bufs=5) as in_pool,
            tc.tile_pool(name="xout_pool", bufs=4) as out_pool,
            tc.tile_pool(name="mpsum_pool", bufs=8, space="PSUM") as mpsum_pool,
        ):
            mw_f = cpool.tile([D, A * A * D], F32)
            nc.sync.dma_start(out=mw_f, in_=m_ext[:, :])
            # Walrus requires f32r matmul operands to be produced as f32r;
            # the ACT copy performs the rounding cast.
            mw_r = cpool.tile([D, A * A * D], F32R)
            nc.scalar.copy(out=mw_r, in_=mw_f)

            for c in range(NCHUNK):
                b0 = c * CHUNK
                # [128 (d), A * 2048]; per partition: one 8 KiB contiguous
                # DRAM run per agent.
                xin = in_pool.tile([128, A * CHUNK], F32R, tag="xin")
                src = x_ext[c].rearrange("a d w -> d a w")
                nc.sync.dma_start(
                    out=xin.rearrange("p (a f) -> p a f", a=A), in_=src
                )

                for h in range(2):
                    # Per-half-chunk output tile: its store DMA (issued on
                    # the otherwise-idle GPSIMD sequencer) waits only on this
                    # half's 8 evacuations, so the SP sequencer's load stream
                    # never blocks behind store data dependencies, and
                    # stores start draining early.
                    xoh = out_pool.tile([128, A * 8 * D], F32, tag="xout")
                    xoh4 = xoh.rearrange("p (a w d) -> p a w d", a=A, d=D)

                    for w4 in range(8):
                        w = 8 * h + w4
                        ps = mpsum_pool.tile([128, A * D], F32, tag="ps")
                        ps_r = ps.rearrange("p (i d) -> p i d", d=D)
                        for j in range(A):
                            nc.tensor.matmul(
                                ps,
                                lhsT=xin[:, j * CHUNK + w * D:
                                         j * CHUNK + (w + 1) * D],
                                rhs=mw_r[:, j * A * D:(j + 1) * A * D],
                                start=(j == 0),
                                stop=(j == A - 1),
                            )
                        # PSUM -> SBUF evacuation, alternating DVE / ACT to
                        # split the copy bandwidth across both engines.
                        if w4 % 2 == 0:
                            nc.vector.tensor_copy(
                                out=xoh4[:, :, w4, :], in_=ps_r
                            )
                        else:
                            nc.scalar.copy(out=xoh4[:, :, w4, :], in_=ps_r)

                    dst = y_ext[:, b0:b0 + CHUNK, :].rearrange(
                        "a (p w) d -> p a w d", p=128
                    )[:, :, 8 * h:8 * h + 8, :]
                    nc.gpsimd.dma_start(out=dst, in_=xoh4)

    # Bacc defers register allocation to its compile() pass (run by
    # finalize); the PJRT exec path serializes nc as-is, so finalize here.
    nc.finalize()
    return nc


def _prep_shard(shard):
    """[A, BC, D] batch-major -> [NCHUNK, A, D, CHUNK] feature-major,
    chunk-major, with within-chunk column order (w, k) -> batch row
    c*2048 + k*16 + w, so matmul PSUM partition order (k) matches the
    batch-major store layout (partition k holds rows b0 + 16k + w)."""
    xs = shard.reshape(A, NCHUNK, 128, W_PER, D)      # [a, c, k, w, d]
    xth = xs.transpose(1, 0, 4, 3, 2)                  # [c, a, d, w, k]
    return np.ascontiguousarray(xth.reshape(NCHUNK, A, D, CHUNK))


def run(inputs, trace=False):
    """Build, compile, and run on 8 cores. Returns (full_output, results_obj)."""
    agent_states = np.asarray(inputs["agent_states"], dtype=np.float32)
    W = np.asarray(inputs["W"], dtype=np.float32)
    b = np.asarray(inputs["b"], dtype=np.float32)

    wp = (W * (1.0 / (A - 1))).astype(np.float32)
    eye = np.eye(D, dtype=np.float32)
    # m[:, j*A*D + i*D : ...] = M_ji  (I on the diagonal -> residual x_i)
    blocks = []
    for j in range(A):
        for i in range(A):
            blocks.append(eye if i == j else wp)
    m_host = np.ascontiguousarray(np.concatenate(blocks, axis=1))

    nc = build_bass()

    in_maps = []
    for i in range(NCORES):
        shard = np.ascontiguousarray(agent_states[:, i * BC:(i + 1) * BC, :])
        in_maps.append({"x": _prep_shard(shard), "m": m_host})

    res = run_bass_kernel_spmd(nc, in_maps, list(range(NCORES)), trace=trace)

    out = np.concatenate([r["y"] for r in res.results], axis=1)
    if np.any(b):
        out = out + b.reshape(1, 1, D)
    return out, res


def kernel(**inputs):
    out, _ = run(inputs, trace=False)
    return out
